# revision 10
# baseline (speedup 1.0000x reference)
"""Trainium2 Bass kernel for the A3TGCN-2-points model (8 NeuronCores, data-parallel).

Math notes (verified against the reference numerically):
  - The reference passes H=None each period, so H0 = 0.  With H0 = 0 the GRU
    reset gate R is multiplied by H0 and vanishes; Z and Htilde only use the
    first HID rows of L_z / L_h.  A period's cell output is
        out(X) = (1 - sigmoid(A X Wz Lz1 + beta_z)) * tanh(A X Wh Lh1 + beta_h)
    where A is the gcn-normalized adjacency (with self loops),
    beta_g = b_g @ Lg1 + lb_g.
  - x_temporal only takes two values per sample: "admission" columns before
    t < LOS and "discharge" columns after.  The attention-weighted scan
    collapses to
        H = w * out(ad) + (1 - w) * out(dis),   w = cumsum(softmax(att))[LOS]
  - pooled = mean over the 32 nodes; logits = relu(pooled@Wc1+bc1)@Wc2+bc2.

Device mapping per core (64 samples):
  - one dma_gather pulls all 64*64 embedding rows (padded to 256B) from HBM
  - row chunks [128 x 32] hit matmul lhsT directly; rhs = I4 (x) A^T applies
    the adjacency per 32-row sample-group AND transposes e onto partitions
  - a single [Mz' | Mh] matmul per 4-chunk batch produces both gate
    pre-activations stacked on 128 partitions; one tanh evaluates both gates
    using sigmoid(-x) = (1 - tanh(x/2)) / 2
  - segmented reduce over nodes, LOS blend, classifier, 64 floats out.
"""

import os
import sys

import numpy as np

sys.path.insert(0, "/opt/trn_rl_repo")

import concourse.bacc as bacc
import concourse.bass as bass
import concourse.mybir as mybir
import concourse.tile as tile
from concourse.bass_utils import run_bass_kernel_spmd

F32 = mybir.dt.float32
I32 = mybir.dt.int32
I16 = mybir.dt.int16
AF = mybir.ActivationFunctionType
ALU = mybir.AluOpType

B, C, N, V, EMB, HID, E, T = 512, 64, 32, 100, 32, 64, 256, 37
NCORES = 8
BSH = B // NCORES            # samples per core
R = BSH * C                  # gathered rows per core (4096)
NCHUNK = R // 128            # 32 row-chunks of 128
NBATCH = NCHUNK // 4         # 8 batches of 4 chunks ([*, 512] tiles)

# blob column layout (weights packed into one [128, 392] f32 DMA)
OWZ, OWH, OLZ, OLH, OWC1 = 0, 64, 128, 192, 256
OBZ, OBH, OLBZ, OLBH, OWC2, OBC1, OBC2, OATT = 384, 385, 386, 387, 388, 389, 390, 391
OID, OION, OIOT = 392, 520, 552
BLOBF = 553


def _install_ntff_hook():
    """The agent image's antenv lacks axon_hooks; synthesize it so trace=True
    can drive NTFF profiling via ctypes on libaxon_pjrt.so (mirrors the
    boot-side hook in trn_boot.py)."""
    import contextlib
    import ctypes
    import types

    if "antenv.axon_hooks" in sys.modules:
        return
    so_path = "/opt/axon/libaxon_pjrt.so"
    mod = types.ModuleType("antenv.axon_hooks")
    state = {"hook": None}

    def set_axon_ntff_profile_hook(h):
        state["hook"] = h

    def get_axon_ntff_profile_hook():
        return state["hook"]

    mod.set_axon_ntff_profile_hook = set_axon_ntff_profile_hook
    mod.get_axon_ntff_profile_hook = get_axon_ntff_profile_hook
    sys.modules["antenv.axon_hooks"] = mod
    try:
        import antenv
        antenv.axon_hooks = mod
    except ImportError:
        pass

    if not os.path.exists(so_path):
        return
    lib = ctypes.CDLL(so_path)
    if not hasattr(lib, "axon_start_nrt_profile"):
        return
    lib.axon_start_nrt_profile.argtypes = [ctypes.POINTER(ctypes.c_int64), ctypes.c_size_t]
    lib.axon_start_nrt_profile.restype = ctypes.c_int64
    lib.axon_stop_nrt_profile.argtypes = [ctypes.c_char_p]
    lib.axon_stop_nrt_profile.restype = ctypes.c_int64

    @contextlib.contextmanager
    def _hook(output_dir, device_ids):
        import jax
        jax.devices()
        if device_ids:
            ids = (ctypes.c_int64 * len(device_ids))(*device_ids)
            rc = lib.axon_start_nrt_profile(ids, len(device_ids))
        else:
            rc = lib.axon_start_nrt_profile(None, 0)
        if rc != 0:
            raise RuntimeError(f"axon_start_nrt_profile rc={rc}")
        try:
            yield
        finally:
            n = lib.axon_stop_nrt_profile(str(output_dir).encode())
            print(f"profile: {n} file(s) written to {output_dir}", file=sys.stderr)

    set_axon_ntff_profile_hook(_hook)


_CACHE = {}
LAST_EXEC_NS = None


def _build_nc():
    nc = bacc.Bacc("TRN2")

    tp = nc.declare_dram_parameter("tp", [C * V, 64], F32, isOutput=False)
    gidx = nc.declare_dram_parameter("gidx", [128, R // 16], I16, isOutput=False)
    edge = nc.declare_dram_parameter("edge", [2, E], I32, isOutput=False)
    los = nc.declare_dram_parameter("los", [1, BSH], I32, isOutput=False)
    blob = nc.declare_dram_parameter("blob", [128, BLOBF], F32, isOutput=False)
    out = nc.declare_dram_parameter("out", [1, BSH], F32, isOutput=True)

    with tile.TileContext(nc) as tc:
        with (
            tc.tile_pool(name="const", bufs=1) as cp,
            tc.tile_pool(name="work", bufs=3) as wp,
            tc.tile_pool(name="ppY", bufs=2, space="PSUM") as ppY,
            tc.tile_pool(name="ppS", bufs=2, space="PSUM") as ppS,
            tc.tile_pool(name="ppP", bufs=1, space="PSUM") as ppP,
            tc.tile_pool(name="ppA", bufs=1, space="PSUM") as ppA,
        ):
            # ---------------- input DMAs ----------------
            blob_sb = cp.tile([128, BLOBF], F32)
            nc.sync.dma_start(out=blob_sb[:], in_=blob[:])
            gsb = cp.tile([128, R // 16], I16)
            nc.sync.dma_start(out=gsb[:], in_=gidx[:])
            esrc = cp.tile([128, 2], I32)
            nc.sync.dma_start(out=esrc[:], in_=edge[0].rearrange("(k p) -> p k", p=128))
            edst = cp.tile([128, 2], I32)
            nc.sync.dma_start(out=edst[:], in_=edge[1].rearrange("(k p) -> p k", p=128))
            los_sb = cp.tile([1, BSH], I32)
            nc.sync.dma_start(out=los_sb[:], in_=los[:])

            def bcol(off, rows=64):
                return blob_sb[0:rows, off:off + 1]

            # ---------------- embedding gather ----------------
            # the SWDGE descriptor ring tops out between 1k and 2k entries per
            # shot, so issue 4 gathers of 1024 rows on separate queues
            xg = cp.tile([128, NCHUNK, 64], F32)
            GCH = 1024
            for g in range(R // GCH):
                nc.gpsimd.dma_gather(
                    out_ap=xg[:, (GCH // 128) * g:(GCH // 128) * (g + 1), :],
                    in_ap=tp[:],
                    idxs_ap=gsb[:, (GCH // 16) * g:(GCH // 16) * (g + 1)],
                    num_idxs=GCH,
                    num_idxs_reg=GCH,
                    elem_size=64,
                )

            # ---------------- constants ----------------
            id128 = blob_sb[:, OID:OID + 128]
            ones_col = cp.tile([128, 1], F32)
            nc.vector.memset(ones_col[:], 1.0)
            ones_row = cp.tile([1, 128], F32)
            nc.vector.memset(ones_row[:], 1.0)
            iota_nf = cp.tile([128, N], F32)
            _src = blob[0, OION:OION + N]
            nc.sync.dma_start(out=iota_nf[:], in_=bass.AP(_src.tensor, _src.offset, [[0, 128]] + list(_src.ap)))

            # ---------------- adjacency build: BD = I4 (x) A^T ----------------
            srcf = cp.tile([128, 2], F32)
            nc.vector.tensor_copy(srcf[:], esrc[:])
            dstf = cp.tile([128, 2], F32)
            nc.vector.tensor_copy(dstf[:], edst[:])

            Dk, Sk = [], []
            for k in range(2):
                d = cp.tile([128, N], F32, tag=f"dk{k}")
                nc.vector.tensor_tensor(
                    out=d[:], in0=dstf[:, k:k + 1].to_broadcast([128, N]),
                    in1=iota_nf[:], op=ALU.is_equal)
                s = cp.tile([128, N], F32, tag=f"sk{k}")
                nc.vector.tensor_tensor(
                    out=s[:], in0=srcf[:, k:k + 1].to_broadcast([128, N]),
                    in1=iota_nf[:], op=ALU.is_equal)
                Dk.append(d)
                Sk.append(s)

            deg_ps = ppP.tile([1, N], F32, tag="prep")
            nc.tensor.matmul(deg_ps[:], ones_col[:], Dk[0][:], start=True, stop=False)
            nc.tensor.matmul(deg_ps[:], ones_col[:], Dk[1][:], start=False, stop=True)
            degp1 = cp.tile([1, N], F32)
            nc.scalar.activation(degp1[:], deg_ps[:], AF.Identity, bias=1.0)
            rec = cp.tile([1, N], F32)
            nc.vector.reciprocal(rec[:], degp1[:])
            dinv_row = cp.tile([1, N], F32)
            nc.scalar.activation(dinv_row[:], rec[:], AF.Sqrt)

            dinvb_ps = ppP.tile([128, N], F32, tag="prep")
            nc.tensor.matmul(dinvb_ps[:], ones_row[:], dinv_row[:], start=True, stop=True)
            dinvb = cp.tile([128, N], F32)
            nc.vector.tensor_copy(dinvb[:], dinvb_ps[:])

            at_ps = ppA.tile([N, N], F32)
            for k in range(2):
                tmp = cp.tile([128, N], F32, tag="degtmp")
                nc.vector.tensor_tensor(out=tmp[:], in0=Dk[k][:], in1=dinvb[:], op=ALU.mult)
                dd = cp.tile([128, 1], F32, tag="ddk")
                nc.vector.tensor_reduce(dd[:], tmp[:], axis=mybir.AxisListType.X, op=ALU.add)
                nc.vector.tensor_tensor(out=tmp[:], in0=Sk[k][:], in1=dinvb[:], op=ALU.mult)
                ds_ = cp.tile([128, 1], F32, tag="dsk")
                nc.vector.tensor_reduce(ds_[:], tmp[:], axis=mybir.AxisListType.X, op=ALU.add)
                nrm = cp.tile([128, 1], F32, tag="nrmk")
                nc.vector.tensor_tensor(out=nrm[:], in0=dd[:], in1=ds_[:], op=ALU.mult)
                sn = cp.tile([128, N], F32, tag=f"snk{k}")
                nc.vector.tensor_scalar(out=sn[:], in0=Sk[k][:], scalar1=nrm[:, :1],
                                        scalar2=None, op0=ALU.mult)
                nc.tensor.matmul(at_ps[:], sn[:], Dk[k][:], start=(k == 0), stop=False)
            diagd = cp.tile([N, N], F32)
            nc.vector.tensor_tensor(out=diagd[:], in0=id128[:N, :N], in1=dinvb[:N, :],
                                    op=ALU.mult)
            nc.tensor.matmul(at_ps[:], diagd[:], diagd[:], start=False, stop=True)

            # engines are lane-locked (no partition shifts), so place the four
            # diagonal blocks with SBUF->SBUF DMAs
            at_sb = cp.tile([N, N], F32)
            nc.vector.tensor_copy(at_sb[:], at_ps[:])
            BD = cp.tile([128, 128], F32)
            nc.vector.memset(BD[:], 0.0)
            for q in range(4):
                nc.sync.dma_start(out=BD[32 * q:32 * (q + 1), 32 * q:32 * (q + 1)],
                                  in_=at_sb[:])

            # ---------------- fused gate weights Mzh = [-Mz/2 | Mh] ----------------
            mzh = cp.tile([EMB, 128], F32)
            betas = []
            for gi, (ow, ob, olb, olg, scale) in enumerate((
                    (OWZ, OBZ, OLBZ, OLZ, -0.5), (OWH, OBH, OLBH, OLH, 1.0))):
                wT_ps = ppP.tile([HID, EMB], F32, tag="prep")
                nc.tensor.transpose(wT_ps[:], blob_sb[0:EMB, ow:ow + HID], id128[:EMB, :EMB])
                wT = cp.tile([HID, EMB], F32, tag=f"wt{gi}")
                nc.vector.tensor_copy(wT[:], wT_ps[:])
                m_ps = ppP.tile([EMB, HID], F32, tag="prep")
                nc.tensor.matmul(m_ps[:], wT[:], blob_sb[0:HID, olg:olg + HID],
                                 start=True, stop=True)
                nc.scalar.activation(mzh[:, 64 * gi:64 * (gi + 1)], m_ps[:], AF.Copy,
                                     scale=scale)
                # beta_g = Lg1^T b_g + lb_g  (as a column), scaled like Mz/Mh
                bb_ps = ppP.tile([HID, 1], F32, tag="prep")
                nc.tensor.matmul(bb_ps[:], blob_sb[0:HID, olg:olg + HID], bcol(ob),
                                 start=True, stop=True)
                bsum = cp.tile([HID, 1], F32, tag=f"bsum{gi}")
                nc.vector.tensor_tensor(out=bsum[:], in0=bb_ps[:], in1=bcol(olb), op=ALU.add)
                bcolg = cp.tile([HID, 1], F32, tag=f"beta{gi}")
                nc.scalar.activation(bcolg[:], bsum[:], AF.Copy, scale=scale)
                betas.append(bcolg)

            # ---------------- LOS blend weights ----------------
            losf = cp.tile([1, BSH], F32)
            nc.vector.tensor_copy(losf[:], los_sb[:])
            losb_ps = ppP.tile([T, BSH], F32, tag="prep")
            nc.tensor.matmul(losb_ps[:], ones_row[:1, :T], losf[:], start=True, stop=True)
            mask = cp.tile([T, BSH], F32)
            nc.vector.tensor_tensor(out=mask[:], in0=blob_sb[0:T, OIOT:OIOT + 1].to_broadcast([T, BSH]),
                                    in1=losb_ps[:], op=ALU.is_lt)
            ecol = cp.tile([T, 1], F32)
            nc.scalar.activation(ecol[:], blob_sb[0:T, OATT:OATT + 1], AF.Exp)
            esum_ps = ppP.tile([1, 1], F32, tag="prep")
            nc.tensor.matmul(esum_ps[:], ecol[:], ones_col[:T, :], start=True, stop=True)
            rinv = cp.tile([1, 1], F32)
            nc.vector.reciprocal(rinv[:], esum_ps[:])
            wraw_ps = ppP.tile([1, BSH], F32, tag="prep")
            nc.tensor.matmul(wraw_ps[:], ecol[:], mask[:], start=True, stop=True)
            wrow = cp.tile([1, BSH], F32)
            nc.vector.tensor_scalar(out=wrow[:], in0=wraw_ps[:], scalar1=rinv[:, :1],
                                    scalar2=None, op0=ALU.mult)
            wb_ps = ppP.tile([HID, BSH], F32, tag="prep")
            nc.tensor.matmul(wb_ps[:], ones_row[:1, :HID], wrow[:], start=True, stop=True)
            wb = cp.tile([HID, BSH], F32)
            nc.vector.tensor_copy(wb[:], wb_ps[:])

            # ---------------- main loop ----------------
            sums = cp.tile([HID, 2 * BSH], F32)
            for jb in range(NBATCH):
                y_ps = ppY.tile([EMB, 512], F32)
                for jj in range(4):
                    j = 4 * jb + jj
                    nc.tensor.matmul(y_ps[:, 128 * jj:128 * (jj + 1)],
                                     xg[:, j, 0:EMB], BD[:], start=True, stop=True)
                ysb = wp.tile([EMB, 512], F32)
                nc.vector.tensor_copy(ysb[:], y_ps[:])
                # z- and h-gate pre-activations side by side on the SAME
                # partitions (engines cannot shift lanes)
                s_ps = ppS.tile([HID, 1024], F32)
                nc.tensor.matmul(s_ps[:, 0:512], mzh[:, 0:64], ysb[:],
                                 start=True, stop=True)
                nc.tensor.matmul(s_ps[:, 512:1024], mzh[:, 64:128], ysb[:],
                                 start=True, stop=True)
                u = wp.tile([HID, 1024], F32)
                nc.scalar.activation(u[:, 0:512], s_ps[:, 0:512], AF.Tanh,
                                     bias=betas[0][:, :1])
                nc.scalar.activation(u[:, 512:1024], s_ps[:, 512:1024], AF.Tanh,
                                     bias=betas[1][:, :1])
                w1 = wp.tile([HID, 512], F32)
                nc.vector.tensor_tensor(out=w1[:], in0=u[:, 0:512], in1=u[:, 512:1024],
                                        op=ALU.mult)
                w2 = wp.tile([HID, 512], F32)
                nc.vector.tensor_tensor(out=w2[:], in0=w1[:], in1=u[:, 512:1024],
                                        op=ALU.add)
                nc.vector.tensor_reduce(
                    sums[:, 16 * jb:16 * (jb + 1)],
                    w2[:].rearrange("p (g n) -> p g n", n=N),
                    axis=mybir.AxisListType.X, op=ALU.add)

            # ---------------- blend + pool + classifier ----------------
            s3 = sums[:].rearrange("p (s k) -> p s k", k=2)
            t1 = cp.tile([HID, BSH], F32)
            nc.vector.tensor_tensor(out=t1[:], in0=s3[:, :, 0], in1=s3[:, :, 1],
                                    op=ALU.subtract)
            t2 = cp.tile([HID, BSH], F32)
            nc.vector.tensor_tensor(out=t2[:], in0=t1[:], in1=wb[:], op=ALU.mult)
            pt = cp.tile([HID, BSH], F32)
            nc.vector.tensor_tensor(out=pt[:], in0=t2[:], in1=s3[:, :, 1], op=ALU.add)

            u1_ps = ppP.tile([2 * HID, BSH], F32, tag="prep")
            nc.tensor.matmul(u1_ps[:], blob_sb[0:HID, OWC1:OWC1 + 2 * HID], pt[:],
                             start=True, stop=True)
            v = cp.tile([2 * HID, BSH], F32)
            nc.scalar.activation(v[:], u1_ps[:], AF.Relu, bias=bcol(OBC1, 128),
                                 scale=1.0 / 64.0)
            y_ps2 = ppP.tile([1, BSH], F32, tag="prep")
            nc.tensor.matmul(y_ps2[:], blob_sb[0:128, OWC2:OWC2 + 1], v[:],
                             start=True, stop=True)
            yrow = cp.tile([1, BSH], F32)
            nc.scalar.activation(yrow[:], y_ps2[:], AF.Identity, bias=bcol(OBC2, 1))
            nc.sync.dma_start(out=out[:], in_=yrow[:])

    nc.finalize()
    return nc


def _stage(inputs):
    """Host-side staging: shard + pack.  Pure layout work, no model math."""
    x_batch = np.asarray(inputs["x_batch"]).astype(np.int32)
    los = np.asarray(inputs["LOS_batch"]).astype(np.int32)
    edge = np.asarray(inputs["template_edge_index"]).astype(np.int32)
    emb = np.asarray(inputs["emb_table"], dtype=np.float32)

    tp = np.zeros((C * V, 64), np.float32)
    tp[:, :EMB] = emb.reshape(C * V, EMB)

    blob = np.zeros((128, BLOBF), np.float32)
    blob[0:EMB, OWZ:OWZ + HID] = inputs["W_z"]
    blob[0:EMB, OWH:OWH + HID] = inputs["W_h"]
    blob[0:HID, OLZ:OLZ + HID] = np.asarray(inputs["L_z"])[:HID]
    blob[0:HID, OLH:OLH + HID] = np.asarray(inputs["L_h"])[:HID]
    blob[0:HID, OWC1:OWC1 + 2 * HID] = inputs["Wc1"]
    blob[0:HID, OBZ] = inputs["b_z"]
    blob[0:HID, OBH] = inputs["b_h"]
    blob[0:HID, OLBZ] = inputs["lb_z"]
    blob[0:HID, OLBH] = inputs["lb_h"]
    blob[0:2 * HID, OWC2] = np.asarray(inputs["Wc2"])[:, 0]
    blob[0:2 * HID, OBC1] = inputs["bc1"]
    blob[0, OBC2] = np.asarray(inputs["bc2"])[0]
    blob[0:T, OATT] = inputs["att"]
    blob[:, OID:OID + 128] = np.eye(128, dtype=np.float32)
    blob[0, OION:OION + N] = np.arange(N, dtype=np.float32)
    blob[0:T, OIOT] = np.arange(T, dtype=np.float32)

    col_off = (np.arange(C, dtype=np.int32) * V)[None, :]
    in_maps = []
    for i in range(NCORES):
        xs = x_batch[i * BSH:(i + 1) * BSH]            # [64, 64]
        flat = (xs + col_off).astype(np.int16).ravel()  # row r = b*64+c
        wrapped = np.tile(flat.reshape(R // 16, 16).T, (8, 1)).copy()  # [128, R//16]
        in_maps.append({
            "tp": tp,
            "gidx": wrapped,
            "edge": edge,
            "los": los[i * BSH:(i + 1) * BSH].reshape(1, BSH).copy(),
            "blob": blob,
        })
    return in_maps


def kernel(**inputs) -> np.ndarray:
    global LAST_EXEC_NS
    if "nc" not in _CACHE:
        _CACHE["nc"] = _build_nc()
    nc = _CACHE["nc"]
    in_maps = _stage(inputs)
    trace = bool(int(os.environ.get("BASSKERNEL_TRACE", "0")))
    kw = {}
    if trace:
        _install_ntff_hook()
        kw["trace"] = True
        tmpdir = os.environ.get("BASSKERNEL_TMPDIR")
        if tmpdir:
            kw["tmpdir"] = tmpdir
    res = run_bass_kernel_spmd(nc, in_maps, core_ids=list(range(NCORES)), **kw)
    LAST_EXEC_NS = getattr(res, "exec_time_ns", None)
    out = np.empty((B, 1), np.float32)
    for i in range(NCORES):
        out[i * BSH:(i + 1) * BSH, 0] = np.asarray(res.results[i]["out"]).reshape(BSH)
    return out
